# revision 3
# baseline (speedup 1.0000x reference)
"""Trainium2 Bass kernel for multi-head cross-attention.

Reference computation (fp32):
  q = x @ Wq; k = ctx @ Wk; v = ctx @ Wv              (per batch)
  sim = einsum('bihd,bjhd->bhij', q, k) * 1/sqrt(64)
  out = softmax(sim) @ v ; out = out @ Wo + bo

Shapes: x (4, 2048, 1024), context (4, 2048, 768), HEADS=8, DIM_HEAD=64.

Sharding: 8 cores = (batch b = core//2) x (query half = core%2). Each core
computes the full attention for its 1024 query rows across all 8 heads with
replicated weights; outputs concatenate — no cross-core reduction.

On-core dataflow. Matmul operands are bf16 (1 cycle/row on the PE; fp32r
measures ~2 cycles/row on TRN2 HW) with fp32 PSUM accumulation:
  - x^T and ctx^T are prepared host-side (feature dim on partitions),
    pre-cast to bf16 on host along with the weights.
  - q^T[c,i], k^T[c,j]  via lhsT=W, rhs=x^T/ctx^T   (feature-major outputs)
  - v[j,c]              via lhsT=ctx^T, rhs=Wv       (context-major output),
    stored per head with an extra ones column: [v_h | 1] (65 cols/head)
  - per head: S^T[j,i] = k_h @ q_h^T  (lhsT=k^T slice, rhs=q^T)
    exp on ACT with scale=1/8 folded in, bf16 out; PV matmul lhsT=[v_h|1]
    accumulates O'[0:64]=unnormalized attn out (transposed) and
    O'[64]=softmax denominator, in one fp32 PSUM accumulation group.
  - normalize: O' is copied to SBUF right away (frees its PSUM slot so the
    next head's PV can start -> the PE never idles long enough for the HAM
    clock gate to re-throttle). recip(denom) on DVE; broadcast over the 64
    partitions via a K=1 fp32r matmul with a ones vector into a shared-pool
    PSUM tile; elementwise mult reads one SBUF + one PSUM operand. Odd heads
    are lane-shifted into the stacked O^T layout via a SBUF->SBUF DMA (DVE
    is lane-locked).
  - final: F = O^T.T @ Wo + ones^T @ bo (bias via K=1 matmul into the same
    PSUM accumulation group).
"""

import ml_dtypes
import numpy as np

import concourse.bass as bass
import concourse.tile as tile
from concourse import bacc, mybir
from concourse.bass_utils import run_bass_kernel_spmd

F32 = mybir.dt.float32
F32R = mybir.dt.float32r
BF16 = mybir.dt.bfloat16

B = 4
NQ_FULL = 2048
NQ = 1024  # local query rows per core
NC = 2048
DQ = 1024
DC = 768
H = 8
DH = 64
INNER = H * DH  # 512
SCALE = DH ** -0.5

AT = DQ // 128   # 8  k-tiles of the q-projection contraction
BT = DC // 128   # 6  k-tiles of the k/v-projection contraction
CT = INNER // 128  # 4 feature tiles of q^T/k^T/o^T
IB = NQ // 128   # 8  query-row blocks
JB = NC // 128   # 16 context-row blocks

_CACHE = {}


def _build_program():
    nc = bacc.Bacc(
        "TRN2",
        target_bir_lowering=False,
        debug=False,
        enable_asserts=False,
    )

    xT = nc.dram_tensor("xT", [DQ, NQ], BF16, kind="ExternalInput").ap()
    ctxT = nc.dram_tensor("ctxT", [DC, NC], BF16, kind="ExternalInput").ap()
    wq = nc.dram_tensor("Wq", [DQ, INNER], BF16, kind="ExternalInput").ap()
    wk = nc.dram_tensor("Wk", [DC, INNER], BF16, kind="ExternalInput").ap()
    wv = nc.dram_tensor("Wv", [DC, INNER], BF16, kind="ExternalInput").ap()
    wo = nc.dram_tensor("Wo", [INNER, DQ], BF16, kind="ExternalInput").ap()
    bo = nc.dram_tensor("bo", [DQ], BF16, kind="ExternalInput").ap()
    out = nc.dram_tensor("out", [NQ, DQ], F32, kind="ExternalOutput").ap()

    with tile.TileContext(nc) as tc:
        with nc.allow_low_precision(reason="bf16 matmul operands"):
            _emit(nc, tc, xT, ctxT, wq, wk, wv, wo, bo, out)

    nc.compile()
    return nc


def _emit(nc, tc, xT, ctxT, wq, wk, wv, wo, bo, out):
    from contextlib import ExitStack

    with ExitStack() as ctx:
        const = ctx.enter_context(tc.tile_pool(name="const", bufs=1))
        persist = ctx.enter_context(tc.tile_pool(name="persist", bufs=1))
        expp = ctx.enter_context(tc.tile_pool(name="expp", bufs=4))
        opool = ctx.enter_context(tc.tile_pool(name="opool", bufs=3))
        rpool = ctx.enter_context(tc.tile_pool(name="rpool", bufs=2))
        otmp = ctx.enter_context(tc.tile_pool(name="otmp", bufs=2))
        outp = ctx.enter_context(tc.tile_pool(name="outp", bufs=2))
        ps_a = ctx.enter_context(tc.tile_pool(name="ps_a", bufs=2, space="PSUM"))
        ps_o = ctx.enter_context(tc.tile_pool(name="ps_o", bufs=2, space="PSUM"))

        # --- constants ---
        bo_sb = const.tile([1, DQ], BF16)
        nc.sync.dma_start(out=bo_sb, in_=bo.unsqueeze(0))
        onesF = const.tile([128, 128], F32)
        nc.vector.memset(onesF, 1.0)
        ones65 = const.tile([65, DH], F32R)  # row 64 = bcast-matmul lhsT
        nc.vector.tensor_copy(ones65, onesF[0:65, 0:DH])
        ones1 = const.tile([1, 128], BF16)  # bias-matmul lhsT
        nc.vector.tensor_copy(ones1, onesF[0:1, :])

        # --- persistent feature-major activations ---
        qT_sb = persist.tile([128, CT, NQ], BF16)
        kT_sb = persist.tile([128, CT, NC], BF16)
        v_sb = persist.tile([128, JB, H * 65], BF16)  # [v_h | 1] per head
        oT_sb = persist.tile([128, CT, NQ], BF16)

        v4 = v_sb.rearrange("p j (h e) -> p j h e", e=65)
        for jb in range(JB):
            nc.vector.tensor_copy(v4[:, jb, :, 64:65], onesF[:, 0:H].unsqueeze(-1))

        # --- phase A: q^T = (x @ Wq)^T via lhsT=Wq, rhs=x^T ---
        with tc.tile_pool(name="phA", bufs=1) as phA:
            xT_sb = phA.tile([128, AT, NQ], BF16)
            nc.sync.dma_start(out=xT_sb, in_=xT.rearrange("(t p) i -> p t i", p=128))
            wq_sb = phA.tile([128, AT, INNER], BF16)
            nc.sync.dma_start(out=wq_sb, in_=wq.rearrange("(t p) c -> p t c", p=128))
            for t in range(CT):
                ps = ps_a.tile([128, NQ], F32, tag="pa")
                for a in range(AT):
                    for ch in range(2):
                        nc.tensor.matmul(
                            ps[:, ch * 512:(ch + 1) * 512],
                            lhsT=wq_sb[:, a, t * 128:(t + 1) * 128],
                            rhs=xT_sb[:, a, ch * 512:(ch + 1) * 512],
                            start=(a == 0),
                            stop=(a == AT - 1),
                        )
                nc.vector.tensor_copy(qT_sb[:, t, :], ps)

        # --- phase B: k^T and v from streamed ctx^T quarters ---
        with tc.tile_pool(name="phBw", bufs=1) as phBw:
            wk_sb = phBw.tile([128, BT, INNER], BF16)
            nc.sync.dma_start(out=wk_sb, in_=wk.rearrange("(t p) c -> p t c", p=128))
            wv_sb = phBw.tile([128, BT, INNER], BF16)
            nc.sync.dma_start(out=wv_sb, in_=wv.rearrange("(t p) c -> p t c", p=128))
            ctxTr = ctxT.rearrange("(t p) j -> p t j", p=128)
            with tc.tile_pool(name="phBx", bufs=2) as phBx:
                for jq in range(4):
                    cx = phBx.tile([128, BT, 512], BF16, tag="cx")
                    nc.sync.dma_start(
                        out=cx, in_=ctxTr[:, :, jq * 512:(jq + 1) * 512]
                    )
                    for t in range(CT):
                        ps = ps_a.tile([128, NQ], F32, tag="pa")
                        for b in range(BT):
                            nc.tensor.matmul(
                                ps[:, 0:512],
                                lhsT=wk_sb[:, b, t * 128:(t + 1) * 128],
                                rhs=cx[:, b, :],
                                start=(b == 0),
                                stop=(b == BT - 1),
                            )
                        nc.vector.tensor_copy(
                            kT_sb[:, t, jq * 512:(jq + 1) * 512], ps[:, 0:512]
                        )
                    for q in range(4):
                        jb = jq * 4 + q
                        ps = ps_a.tile([128, NQ], F32, tag="pa")
                        for b in range(BT):
                            nc.tensor.matmul(
                                ps[:, 0:512],
                                lhsT=cx[:, b, q * 128:(q + 1) * 128],
                                rhs=wv_sb[:, b, :],
                                start=(b == 0),
                                stop=(b == BT - 1),
                            )
                        nc.vector.tensor_copy(
                            v4[:, jb, :, 0:64],
                            ps[:, 0:512].rearrange("p (h d) -> p h d", d=DH),
                        )

        # --- attention per head ---
        for h in range(H):
            t, po = h // 2, 64 * (h % 2)
            qTh = qT_sb[po:po + 64, t, :]
            kTh = kT_sb[po:po + 64, t, :]
            ops = ps_o.tile([128, NQ], F32, tag="po")  # rows 0-63 O'; row 64 denom
            for jb in range(JB):
                sps = ps_a.tile([128, NQ], F32, tag="pa")
                for ch in range(2):
                    nc.tensor.matmul(
                        sps[:, ch * 512:(ch + 1) * 512],
                        lhsT=kTh[:, jb * 128:(jb + 1) * 128],
                        rhs=qTh[:, ch * 512:(ch + 1) * 512],
                        start=True,
                        stop=True,
                    )
                es = expp.tile([128, NQ], BF16, tag="es")
                nc.scalar.activation(
                    es, sps, mybir.ActivationFunctionType.Exp, scale=SCALE
                )
                for ch in range(2):
                    nc.tensor.matmul(
                        ops[0:65, ch * 512:(ch + 1) * 512],
                        lhsT=v4[:, jb, h, :],
                        rhs=es[:, ch * 512:(ch + 1) * 512],
                        start=(jb == 0),
                        stop=(jb == JB - 1),
                    )
            # Evacuate O' to SBUF immediately: frees the ps_o slot so the next
            # head's PV can start while this head normalizes.
            osb = opool.tile([65, NQ], F32, tag="osb")
            nc.vector.tensor_copy(osb, ops[0:65, :])
            # normalize: O^T = O'[0:64] * (1/denom) broadcast over partitions
            rt = rpool.tile([65, NQ], F32R, tag="rt")
            nc.vector.reciprocal(rt[64:65, :], osb[64:65, :])
            rb = ps_a.tile([128, NQ], F32, tag="pa")
            for ch in range(2):
                nc.tensor.matmul(
                    rb[0:64, ch * 512:(ch + 1) * 512],
                    lhsT=ones65[64:65, :],
                    rhs=rt[64:65, ch * 512:(ch + 1) * 512],
                    start=True,
                    stop=True,
                )
            if h % 2 == 0:
                nc.vector.tensor_mul(oT_sb[0:64, t, :], osb[0:64, :], rb[0:64, :])
            else:
                ot = otmp.tile([64, NQ], BF16, tag="ot")
                nc.vector.tensor_mul(ot, osb[0:64, :], rb[0:64, :])
                nc.sync.dma_start(out=oT_sb[64:128, t, :], in_=ot)

        # --- output projection: F = O^T.T @ Wo + bias ---
        with tc.tile_pool(name="phD", bufs=1) as phD:
            wo_sb = phD.tile([128, CT, DQ], BF16)
            nc.sync.dma_start(out=wo_sb, in_=wo.rearrange("(t p) e -> p t e", p=128))
            for ib in range(IB):
                fp = ps_a.tile([128, NQ], F32, tag="pa")
                for ch in range(2):
                    for t in range(CT):
                        nc.tensor.matmul(
                            fp[:, ch * 512:(ch + 1) * 512],
                            lhsT=oT_sb[:, t, ib * 128:(ib + 1) * 128],
                            rhs=wo_sb[:, t, ch * 512:(ch + 1) * 512],
                            start=(t == 0),
                            stop=False,
                        )
                    nc.tensor.matmul(
                        fp[:, ch * 512:(ch + 1) * 512],
                        lhsT=ones1,
                        rhs=bo_sb[0:1, ch * 512:(ch + 1) * 512],
                        start=False,
                        stop=True,
                    )
                ost = outp.tile([128, DQ], F32)
                nc.vector.tensor_copy(ost, fp)
                nc.sync.dma_start(out=out[ib * 128:(ib + 1) * 128, :], in_=ost)


def get_program():
    if "nc" not in _CACHE:
        _CACHE["nc"] = _build_program()
    return _CACHE["nc"]


def make_in_maps(x, context, Wq, Wk, Wv, Wo, bo):
    bf = ml_dtypes.bfloat16
    in_maps = []
    wq_b = np.asarray(Wq).astype(bf)
    wk_b = np.asarray(Wk).astype(bf)
    wv_b = np.asarray(Wv).astype(bf)
    wo_b = np.asarray(Wo).astype(bf)
    bo_b = np.asarray(bo).astype(bf)
    for c in range(8):
        b, half = c // 2, c % 2
        in_maps.append({
            "xT": np.ascontiguousarray(
                x[b, half * NQ:(half + 1) * NQ, :].T
            ).astype(bf),
            "ctxT": np.ascontiguousarray(context[b].T).astype(bf),
            "Wq": wq_b,
            "Wk": wk_b,
            "Wv": wv_b,
            "Wo": wo_b,
            "bo": bo_b,
        })
    return in_maps


def kernel(x, context, Wq, Wk, Wv, Wo, bo):
    nc = get_program()
    in_maps = make_in_maps(x, context, Wq, Wk, Wv, Wo, bo)
    res = run_bass_kernel_spmd(nc, in_maps, list(range(8)))
    out = np.empty((B, NQ_FULL, DQ), np.float32)
    for c in range(8):
        b, half = c // 2, c % 2
        out[b, half * NQ:(half + 1) * NQ, :] = res.results[c]["out"]
    return out


# revision 4
# speedup vs baseline: 1.5178x; 1.5178x over previous
"""Trainium2 Bass kernel for multi-head cross-attention.

Reference computation (fp32):
  q = x @ Wq; k = ctx @ Wk; v = ctx @ Wv              (per batch)
  sim = einsum('bihd,bjhd->bhij', q, k) * 1/sqrt(64)
  out = softmax(sim) @ v ; out = out @ Wo + bo

Shapes: x (4, 2048, 1024), context (4, 2048, 768), HEADS=8, DIM_HEAD=64.

Sharding: 8 cores = (batch b = core//2) x (query half = core%2). Each core
computes the full attention for its 1024 query rows across all 8 heads with
replicated weights; outputs concatenate — no cross-core reduction.

On-core dataflow. Matmul operands are bf16 (1 cycle/row on the PE; fp32r
measures ~2 cycles/row on TRN2 HW) with fp32 PSUM accumulation:
  - x^T and ctx^T are prepared host-side (feature dim on partitions),
    pre-cast to bf16 on host along with the weights.
  - q^T[c,i], k^T[c,j]  via lhsT=W, rhs=x^T/ctx^T   (feature-major outputs)
  - v[j,c]              via lhsT=ctx^T, rhs=Wv       (context-major output),
    stored per head with an extra ones column: [v_h | 1] (65 cols/head)
  - per head: S^T[j,i] = k_h @ q_h^T  (lhsT=k^T slice, rhs=q^T)
    exp on ACT with scale=1/8 folded in, bf16 out; PV matmul lhsT=[v_h|1]
    accumulates O'[0:64]=unnormalized attn out (transposed) and
    O'[64]=softmax denominator, in one fp32 PSUM accumulation group.
  - normalize (entirely off the PE so its instruction queue never stalls —
    a PE-visible wait on the reciprocal re-throttles the HAM clock gate):
    O' is copied to SBUF immediately (frees the PSUM slot for the next
    head's PV), recip(denom) on DVE, a lane-shift DMA moves it to
    partition 0, gpsimd partition_broadcast replicates it over 64 lanes,
    and a DVE mult normalizes. Odd heads are lane-shifted into the stacked
    O^T layout via a SBUF->SBUF DMA (DVE is lane-locked).
  - final: F = O^T.T @ Wo + ones^T @ bo (bias via K=1 matmul into the same
    PSUM accumulation group).

Input DMAs are split per 128-row tile so the first projection matmuls
start as soon as their operands land rather than after the full tensor.
"""

import ml_dtypes
import numpy as np

import concourse.bass as bass
import concourse.tile as tile
from concourse import bacc, mybir
from concourse.bass_utils import run_bass_kernel_spmd

F32 = mybir.dt.float32
BF16 = mybir.dt.bfloat16

B = 4
NQ_FULL = 2048
NQ = 1024  # local query rows per core
NC = 2048
DQ = 1024
DC = 768
H = 8
DH = 64
INNER = H * DH  # 512
SCALE = DH ** -0.5

AT = DQ // 128   # 8  k-tiles of the q-projection contraction
BT = DC // 128   # 6  k-tiles of the k/v-projection contraction
CT = INNER // 128  # 4 feature tiles of q^T/k^T/o^T
IB = NQ // 128   # 8  query-row blocks
JB = NC // 128   # 16 context-row blocks

_CACHE = {}


def _build_program():
    nc = bacc.Bacc(
        "TRN2",
        target_bir_lowering=False,
        debug=False,
        enable_asserts=False,
    )

    xT = nc.dram_tensor("xT", [DQ, NQ], BF16, kind="ExternalInput").ap()
    ctxT = nc.dram_tensor("ctxT", [DC, NC], BF16, kind="ExternalInput").ap()
    wq = nc.dram_tensor("Wq", [DQ, INNER], BF16, kind="ExternalInput").ap()
    wk = nc.dram_tensor("Wk", [DC, INNER], BF16, kind="ExternalInput").ap()
    wv = nc.dram_tensor("Wv", [DC, INNER], BF16, kind="ExternalInput").ap()
    wo = nc.dram_tensor("Wo", [INNER, DQ], BF16, kind="ExternalInput").ap()
    bo = nc.dram_tensor("bo", [DQ], BF16, kind="ExternalInput").ap()
    out = nc.dram_tensor("out", [NQ, DQ], F32, kind="ExternalOutput").ap()

    with tile.TileContext(nc) as tc:
        with nc.allow_low_precision(reason="bf16 matmul operands"):
            _emit(nc, tc, xT, ctxT, wq, wk, wv, wo, bo, out)

    nc.compile()
    return nc


def _emit(nc, tc, xT, ctxT, wq, wk, wv, wo, bo, out):
    from contextlib import ExitStack

    with ExitStack() as ctx:
        const = ctx.enter_context(tc.tile_pool(name="const", bufs=1))
        persist = ctx.enter_context(tc.tile_pool(name="persist", bufs=1))
        expp = ctx.enter_context(tc.tile_pool(name="expp", bufs=4))
        opool = ctx.enter_context(tc.tile_pool(name="opool", bufs=3))
        rpool = ctx.enter_context(tc.tile_pool(name="rpool", bufs=2))
        otmp = ctx.enter_context(tc.tile_pool(name="otmp", bufs=2))
        outp = ctx.enter_context(tc.tile_pool(name="outp", bufs=2))
        ps_a = ctx.enter_context(tc.tile_pool(name="ps_a", bufs=2, space="PSUM"))
        ps_o = ctx.enter_context(tc.tile_pool(name="ps_o", bufs=2, space="PSUM"))

        # --- constants ---
        bo_sb = const.tile([1, DQ], BF16)
        nc.sync.dma_start(out=bo_sb, in_=bo.unsqueeze(0))
        onesF = const.tile([128, 128], F32)
        nc.vector.memset(onesF, 1.0)
        ones1 = const.tile([1, 128], BF16)  # bias-matmul lhsT
        nc.vector.tensor_copy(ones1, onesF[0:1, :])

        # --- persistent feature-major activations ---
        qT_sb = persist.tile([128, CT, NQ], BF16)
        kT_sb = persist.tile([128, CT, NC], BF16)
        v_sb = persist.tile([128, JB, H * 65], BF16)  # [v_h | 1] per head
        oT_sb = persist.tile([128, CT, NQ], BF16)
        wo_sb = persist.tile([128, CT, DQ], BF16)
        # Wo prefetch: queued first so it lands long before the projection ends.
        wor = wo.rearrange("(t p) e -> p t e", p=128)
        for t in range(CT):
            nc.sync.dma_start(out=wo_sb[:, t, :], in_=wor[:, t, :])

        v4 = v_sb.rearrange("p j (h e) -> p j h e", e=65)
        for jb in range(JB):
            nc.vector.tensor_copy(v4[:, jb, :, 64:65], onesF[:, 0:H].unsqueeze(-1))

        # --- phase A: q^T = (x @ Wq)^T via lhsT=Wq, rhs=x^T ---
        with tc.tile_pool(name="phA", bufs=1) as phA:
            xT_sb = phA.tile([128, AT, NQ], BF16)
            wq_sb = phA.tile([128, AT, INNER], BF16)
            xTr = xT.rearrange("(t p) i -> p t i", p=128)
            wqr = wq.rearrange("(t p) c -> p t c", p=128)
            for a in range(AT):
                nc.sync.dma_start(out=wq_sb[:, a, :], in_=wqr[:, a, :])
                nc.sync.dma_start(out=xT_sb[:, a, :], in_=xTr[:, a, :])
            for t in range(CT):
                ps = ps_a.tile([128, NQ], F32, tag="pa")
                for a in range(AT):
                    for ch in range(2):
                        nc.tensor.matmul(
                            ps[:, ch * 512:(ch + 1) * 512],
                            lhsT=wq_sb[:, a, t * 128:(t + 1) * 128],
                            rhs=xT_sb[:, a, ch * 512:(ch + 1) * 512],
                            start=(a == 0),
                            stop=(a == AT - 1),
                        )
                nc.vector.tensor_copy(qT_sb[:, t, :], ps)

        # --- phase B: k^T and v from streamed ctx^T quarters ---
        with tc.tile_pool(name="phBw", bufs=1) as phBw:
            wk_sb = phBw.tile([128, BT, INNER], BF16)
            wv_sb = phBw.tile([128, BT, INNER], BF16)
            wkr = wk.rearrange("(t p) c -> p t c", p=128)
            wvr = wv.rearrange("(t p) c -> p t c", p=128)
            for b in range(BT):
                nc.sync.dma_start(out=wk_sb[:, b, :], in_=wkr[:, b, :])
                nc.sync.dma_start(out=wv_sb[:, b, :], in_=wvr[:, b, :])
            ctxTr = ctxT.rearrange("(t p) j -> p t j", p=128)
            with tc.tile_pool(name="phBx", bufs=2) as phBx:
                for jq in range(4):
                    cx = phBx.tile([128, BT, 512], BF16, tag="cx")
                    for b in range(BT):
                        nc.sync.dma_start(
                            out=cx[:, b, :],
                            in_=ctxTr[:, b, jq * 512:(jq + 1) * 512],
                        )
                    for t in range(CT):
                        ps = ps_a.tile([128, NQ], F32, tag="pa")
                        for b in range(BT):
                            nc.tensor.matmul(
                                ps[:, 0:512],
                                lhsT=wk_sb[:, b, t * 128:(t + 1) * 128],
                                rhs=cx[:, b, :],
                                start=(b == 0),
                                stop=(b == BT - 1),
                            )
                        nc.vector.tensor_copy(
                            kT_sb[:, t, jq * 512:(jq + 1) * 512], ps[:, 0:512]
                        )
                    for q in range(4):
                        jb = jq * 4 + q
                        ps = ps_a.tile([128, NQ], F32, tag="pa")
                        for b in range(BT):
                            nc.tensor.matmul(
                                ps[:, 0:512],
                                lhsT=cx[:, b, q * 128:(q + 1) * 128],
                                rhs=wv_sb[:, b, :],
                                start=(b == 0),
                                stop=(b == BT - 1),
                            )
                        nc.vector.tensor_copy(
                            v4[:, jb, :, 0:64],
                            ps[:, 0:512].rearrange("p (h d) -> p h d", d=DH),
                        )

        # --- attention per head ---
        for h in range(H):
            t, po = h // 2, 64 * (h % 2)
            qTh = qT_sb[po:po + 64, t, :]
            kTh = kT_sb[po:po + 64, t, :]
            ops = ps_o.tile([128, NQ], F32, tag="po")  # rows 0-63 O'; row 64 denom
            for jb in range(JB):
                sps = ps_a.tile([128, NQ], F32, tag="pa")
                for ch in range(2):
                    nc.tensor.matmul(
                        sps[:, ch * 512:(ch + 1) * 512],
                        lhsT=kTh[:, jb * 128:(jb + 1) * 128],
                        rhs=qTh[:, ch * 512:(ch + 1) * 512],
                        start=True,
                        stop=True,
                    )
                es = expp.tile([128, NQ], BF16, tag="es")
                nc.scalar.activation(
                    es, sps, mybir.ActivationFunctionType.Exp, scale=SCALE
                )
                for ch in range(2):
                    nc.tensor.matmul(
                        ops[0:65, ch * 512:(ch + 1) * 512],
                        lhsT=v4[:, jb, h, :],
                        rhs=es[:, ch * 512:(ch + 1) * 512],
                        start=(jb == 0),
                        stop=(jb == JB - 1),
                    )
            # Evacuate O' to SBUF immediately: frees the ps_o slot so the next
            # head's PV can start while this head normalizes off the PE.
            osb = opool.tile([65, NQ], F32, tag="osb")
            nc.vector.tensor_copy(osb, ops[0:65, :])
            # normalize: O^T = O'[0:64] * (1/denom) broadcast over partitions
            rt = rpool.tile([65, NQ], F32, tag="rt")
            nc.vector.reciprocal(rt[64:65, :], osb[64:65, :])
            r0 = rpool.tile([1, NQ], F32, tag="r0")
            nc.sync.dma_start(out=r0, in_=rt[64:65, :])  # lane 64 -> lane 0
            rbx = rpool.tile([64, NQ], F32, tag="rbx")
            nc.gpsimd.partition_broadcast(rbx, r0)
            if h % 2 == 0:
                nc.vector.tensor_mul(oT_sb[0:64, t, :], osb[0:64, :], rbx)
            else:
                ot = otmp.tile([64, NQ], BF16, tag="ot")
                nc.vector.tensor_mul(ot, osb[0:64, :], rbx)
                nc.sync.dma_start(out=oT_sb[64:128, t, :], in_=ot)

        # --- output projection: F = O^T.T @ Wo + bias ---
        for ib in range(IB):
            fp = ps_a.tile([128, NQ], F32, tag="pa")
            for ch in range(2):
                for t in range(CT):
                    nc.tensor.matmul(
                        fp[:, ch * 512:(ch + 1) * 512],
                        lhsT=oT_sb[:, t, ib * 128:(ib + 1) * 128],
                        rhs=wo_sb[:, t, ch * 512:(ch + 1) * 512],
                        start=(t == 0),
                        stop=False,
                    )
                nc.tensor.matmul(
                    fp[:, ch * 512:(ch + 1) * 512],
                    lhsT=ones1,
                    rhs=bo_sb[0:1, ch * 512:(ch + 1) * 512],
                    start=False,
                    stop=True,
                )
            ost = outp.tile([128, DQ], F32)
            nc.vector.tensor_copy(ost, fp)
            nc.sync.dma_start(out=out[ib * 128:(ib + 1) * 128, :], in_=ost)


def get_program():
    if "nc" not in _CACHE:
        _CACHE["nc"] = _build_program()
    return _CACHE["nc"]


def make_in_maps(x, context, Wq, Wk, Wv, Wo, bo):
    bf = ml_dtypes.bfloat16
    in_maps = []
    wq_b = np.asarray(Wq).astype(bf)
    wk_b = np.asarray(Wk).astype(bf)
    wv_b = np.asarray(Wv).astype(bf)
    wo_b = np.asarray(Wo).astype(bf)
    bo_b = np.asarray(bo).astype(bf)
    for c in range(8):
        b, half = c // 2, c % 2
        in_maps.append({
            "xT": np.ascontiguousarray(
                x[b, half * NQ:(half + 1) * NQ, :].T
            ).astype(bf),
            "ctxT": np.ascontiguousarray(context[b].T).astype(bf),
            "Wq": wq_b,
            "Wk": wk_b,
            "Wv": wv_b,
            "Wo": wo_b,
            "bo": bo_b,
        })
    return in_maps


def kernel(x, context, Wq, Wk, Wv, Wo, bo):
    nc = get_program()
    in_maps = make_in_maps(x, context, Wq, Wk, Wv, Wo, bo)
    res = run_bass_kernel_spmd(nc, in_maps, list(range(8)))
    out = np.empty((B, NQ_FULL, DQ), np.float32)
    for c in range(8):
        b, half = c // 2, c % 2
        out[b, half * NQ:(half + 1) * NQ, :] = res.results[c]["out"]
    return out
